# revision 1
# baseline (speedup 1.0000x reference)
"""NCC loss (local normalized cross-correlation, window 9^3) on 8 Trainium2
NeuronCores.

Reference: 5 channels [I, J, I^2, J^2, IJ] box-filtered (separable 9-tap mean,
SAME zero-pad) over a 192^3 volume; cc = sigma12^2/(sigma1^2*sigma2^2+eps);
output = 1 - mean(cc).

Sharding: depth axis. Core c computes output slices [24c, 24c+24), reading
padded input slices [24c, 24c+32) of the (+4 both ends) zero-padded volume.
H/W are raw-zero-extended to 200 on the host; prep ops turn the raw zeros
into the correct shifted pad samples, so all filters are pure (unclipped)
banded matmuls and outputs are exactly the interior 192.

Per-core pipeline (validated numerically in model.py, rel err ~6e-5):
  prep   : mean-shift (I-0.5, J-0.5) + products -> 5 bf16 channels/slice,
           [h-part, (ch,w)-free]; h tiles: ext 0..127 and ext 112..199.
  H pass : banded matmuls (TensorE) accumulated over slices into PSUM
           (cumsum over D); bf16 snapshots to SBUF each slice (DVE+ACT).
  D pass : window sum = snapshot difference C[z+8]-C[z-1] (one TT).
  W pass : DMA x-bar transpose of diffs to [w-part, h-free] + banded matmuls.
  cc     : elementwise DVE/ACT; division via exp(ln(num)-ln(den));
           per-partition sums via activation(accum_out).
Host: 1 - sum(partials)/192^3.
"""

import sys

import numpy as np

sys.path.insert(0, "/opt/trn_rl_repo")

import contextlib

import concourse.bacc as bacc
import concourse.mybir as mybir
from concourse import tile
from concourse.bass_utils import run_bass_kernel_spmd

F32 = mybir.dt.float32
BF16 = mybir.dt.bfloat16
AOT = mybir.AluOpType
ACTF = mybir.ActivationFunctionType
AXL = mybir.AxisListType

H = 192
W = 192
D_TOT = 192
HE = 200   # extended h (4 raw-zero pad each side)
WE = 200   # extended w
PAD = 4
N_CORES = 8

HA = 112   # H-pass out: ext rows 4..115  == orig h 0..111
HB = 80    # H-pass out: ext rows 116..195 == orig h 112..191
KT = 128   # chanT partitions: ext-h 0..127
KB = 88    # chanB partitions: ext-h 112..199

BAND_C = 1.0 / 27.0
NCH = 5
FREE = NCH * WE            # 1000 (channel tiles, snapshots)
PIECE = 500                # free elems per matmul (PSUM: placed at 512 offs)
TFREE = NCH * H            # 960 (transposed tiles, F tiles)
TPIECE = 480

EPS = float(np.finfo(np.float32).eps)
TINY = float(np.finfo(np.float32).tiny)


def _band(rows, cols, lo, hi, val):
    k = np.arange(rows)[:, None]
    m = np.arange(cols)[None, :]
    return np.where((k - m >= lo) & (k - m <= hi), val, 0.0).astype(np.float32)


def make_consts():
    import ml_dtypes

    # master upper band, k-m in [0,8]; sliced for all four matmul uses
    return _band(120, 112, 0, 8, BAND_C).astype(ml_dtypes.bfloat16)


def build_program(din, dout):
    assert din == dout + 2 * PAD
    nc = bacc.Bacc(
        "TRN2", target_bir_lowering=False, debug=False, num_devices=N_CORES
    )

    pred_d = nc.dram_tensor("pred", [din, HE, WE], F32, kind="ExternalInput")
    targ_d = nc.dram_tensor("targ", [din, HE, WE], F32, kind="ExternalInput")
    band_d = nc.dram_tensor("band", [120, 112], BF16, kind="ExternalInput")
    out_d = nc.dram_tensor("out", [96, 1], F32, kind="ExternalOutput")

    pred = pred_d.ap()
    targ = targ_d.ap()
    NACC = 2 * dout

    with tile.TileContext(nc) as tc, contextlib.ExitStack() as ctx:
        consts = ctx.enter_context(tc.tile_pool(name="consts", bufs=1))
        raws = ctx.enter_context(tc.tile_pool(name="raws", bufs=3))
        chans = ctx.enter_context(tc.tile_pool(name="chans", bufs=3))
        snaps = ctx.enter_context(tc.tile_pool(name="snaps", bufs=11))
        diffs = ctx.enter_context(tc.tile_pool(name="diffs", bufs=2))
        tts = ctx.enter_context(tc.tile_pool(name="tts", bufs=2))
        fts = ctx.enter_context(tc.tile_pool(name="fts", bufs=2))
        ccs = ctx.enter_context(tc.tile_pool(name="ccs", bufs=2))
        accp = ctx.enter_context(tc.tile_pool(name="accp", bufs=1))
        ps_h = ctx.enter_context(tc.tile_pool(name="psh", bufs=1, space="PSUM"))
        ps_w = ctx.enter_context(tc.tile_pool(name="psw", bufs=1, space="PSUM"))

        band = consts.tile([120, 112], BF16, tag="band")
        nc.sync.dma_start(band[:], band_d.ap())

        bias_nh = consts.tile([128, 1], F32, tag="bias_nh")
        bias_tiny = consts.tile([128, 1], F32, tag="bias_tiny")
        bias_eps = consts.tile([128, 1], F32, tag="bias_eps")
        nc.vector.memset(bias_nh[:], -0.5)
        nc.vector.memset(bias_tiny[:], TINY)
        nc.vector.memset(bias_eps[:], EPS)

        # H-cum PSUM; free padded to 1024 so each 500-piece sits in one bank
        psA = ps_h.tile([HA, 1024], F32, tag="psA")
        psB = ps_h.tile([HB, 1024], F32, tag="psB")
        psA3 = psA.rearrange("p (b w) -> p b w", b=2)  # [*, 2, 512]
        psB3 = psB.rearrange("p (b w) -> p b w", b=2)

        zsnapA = consts.tile([HA, FREE], BF16, tag="zsnapA")
        zsnapB = consts.tile([HB, FREE], BF16, tag="zsnapB")
        nc.vector.memset(zsnapA[:], 0.0)
        nc.vector.memset(zsnapB[:], 0.0)

        acc = accp.tile([96, NACC], F32, tag="acc")
        nc.vector.memset(acc[:], 0.0)

        # Persistent ping-pong diff tiles; free layout [wc:2][ch:5][128] where
        # cols 0..103 of each 128-block hold ext-w 0..103 (wc0) / 96..199
        # (wc1) and cols 104..127 stay zero (memset once) so the x-bar
        # transposes read fully-initialized 128-wide blocks.
        diff_tiles = []
        for pp in range(2):
            dA = diffs.tile(
                [HA, 2 * NCH * 128], BF16, tag=f"dA{pp}", name=f"dA{pp}"
            )
            dB = diffs.tile(
                [HB, 2 * NCH * 128], BF16, tag=f"dB{pp}", name=f"dB{pp}"
            )
            nc.vector.memset(dA[:], 0.0)
            nc.vector.memset(dB[:], 0.0)
            diff_tiles.append((dA, dB))

        snapsA = {}
        snapsB = {}

        def h_pass(z):
            rawT = raws.tile([KT, 2 * WE], F32, tag="rawT", name="rawT")
            rawB = raws.tile([KB, 2 * WE], F32, tag="rawB", name="rawB")
            nc.sync.dma_start(rawT[:, 0:WE], targ[z, 0:KT, :])
            nc.sync.dma_start(rawT[:, WE:], pred[z, 0:KT, :])
            nc.sync.dma_start(rawB[:, 0:WE], targ[z, HE - KB : HE, :])
            nc.sync.dma_start(rawB[:, WE:], pred[z, HE - KB : HE, :])

            chanT = chans.tile([KT, FREE], BF16, tag="chanT", name="chanT")
            chanB = chans.tile([KB, FREE], BF16, tag="chanB", name="chanB")
            for ch, raw in ((chanT, rawT), (chanB, rawB)):
                # ch0 = I-0.5, ch1 = J-0.5
                nc.vector.tensor_scalar_add(ch[:, 0 : 2 * WE], raw[:], -0.5)
                # ch2 = (I-0.5)^2, ch3 = (J-0.5)^2
                nc.scalar.activation(
                    ch[:, 2 * WE : 4 * WE], raw[:], ACTF.Square,
                    bias=bias_nh[0 : ch.shape[0], :],
                )
                # ch4 = (J-0.5)*(I-0.5)
                nc.vector.scalar_tensor_tensor(
                    ch[:, 4 * WE : FREE],
                    raw[:, WE:],
                    -0.5,
                    ch[:, 0:WE],
                    AOT.add,
                    AOT.mult,
                )

            # start only on the first slice (PSUM then accumulates across
            # slices = cumsum over D). stop is a HW no-op; asserting it every
            # slice keeps the simulator's PSUM-read-while-group-open check
            # happy, with skip_group_check for the reopen.
            start = z == 0
            for p in range(2):
                sl = slice(p * PIECE, (p + 1) * PIECE)
                nc.tensor.matmul(
                    psA3[:, p, 0:PIECE], band[0:120, 0:HA], chanT[0:120, sl],
                    start=start, stop=True, skip_group_check=True,
                )
                nc.tensor.matmul(
                    psB3[:, p, 0:PIECE], band[0:KB, 0:HB], chanB[:, sl],
                    start=start, stop=True, skip_group_check=True,
                )

            sA = snaps.tile([HA, FREE], BF16, tag="snapA", name="snapA")
            sB = snaps.tile([HB, FREE], BF16, tag="snapB", name="snapB")
            sA3 = sA.rearrange("p (b w) -> p b w", b=2)
            sB3 = sB.rearrange("p (b w) -> p b w", b=2)
            nc.vector.tensor_copy(sA3[:], psA3[:, :, 0:PIECE])
            nc.scalar.copy(sB3[:], psB3[:, :, 0:PIECE])
            snapsA[z] = sA
            snapsB[z] = sB

        def w_pass(oz):
            hi_A, hi_B = snapsA[oz + 8], snapsB[oz + 8]
            lo_A = zsnapA if oz == 0 else snapsA[oz - 1]
            lo_B = zsnapB if oz == 0 else snapsB[oz - 1]
            snapsA.pop(oz - 2, None)
            snapsB.pop(oz - 2, None)

            # D-filtered slice into the ping-pong diff tiles (valid cols
            # 0..103 per block: wc0 = ext-w 0..103, wc1 = ext-w 96..199)
            dA, dB = diff_tiles[oz % 2]
            for dd, hi, lo in ((dA, hi_A, lo_A), (dB, hi_B, lo_B)):
                d3 = dd.rearrange("p (b c w) -> p b c w", b=2, c=NCH)
                hi3 = hi.rearrange("p (c w) -> p c w", c=NCH)
                lo3 = lo.rearrange("p (c w) -> p c w", c=NCH)
                for wc in range(2):
                    w0 = wc * 96
                    nc.vector.tensor_tensor(
                        d3[:, wc, :, 0:104],
                        hi3[:, :, w0 : w0 + 104],
                        lo3[:, :, w0 : w0 + 104],
                        AOT.subtract,
                    )

            # x-bar transposes: [(HA|HB), 128] -> [128, (HA|HB)] per (wc, ch)
            t0 = tts.tile([128, TFREE], BF16, tag="t0", name="t0")
            t1 = tts.tile([128, TFREE], BF16, tag="t1", name="t1")
            for wc, tt in ((0, t0), (1, t1)):
                for c in range(NCH):
                    src = slice((wc * NCH + c) * 128, (wc * NCH + c + 1) * 128)
                    nc.sync.dma_start_transpose(
                        tt[:, c * H : c * H + HA], dA[:, src]
                    )
                    nc.sync.dma_start_transpose(
                        tt[:, c * H + HA : (c + 1) * H], dB[:, src]
                    )

            pw0 = ps_w.tile([96, 1024], F32, tag="pw0", name="pw0")
            pw1 = ps_w.tile([96, 1024], F32, tag="pw1", name="pw1")
            pw03 = pw0.rearrange("p (b w) -> p b w", b=2)
            pw13 = pw1.rearrange("p (b w) -> p b w", b=2)
            for p in range(2):
                sl = slice(p * TPIECE, (p + 1) * TPIECE)
                nc.tensor.matmul(
                    pw03[:, p, 0:TPIECE], band[0:104, 0:96], t0[0:104, sl],
                    start=True, stop=True,
                )
                nc.tensor.matmul(
                    pw13[:, p, 0:TPIECE], band[0:104, 0:96], t1[0:104, sl],
                    start=True, stop=True,
                )

            f0 = fts.tile([96, TFREE], BF16, tag="f0", name="f0")
            f1 = fts.tile([96, TFREE], BF16, tag="f1", name="f1")
            f03 = f0.rearrange("p (b w) -> p b w", b=2)
            f13 = f1.rearrange("p (b w) -> p b w", b=2)
            nc.vector.tensor_copy(f03[:], pw03[:, :, 0:TPIECE])
            nc.scalar.copy(f13[:], pw13[:, :, 0:TPIECE])

            for fi, ff in enumerate((f0, f1)):
                F_I = ff[:, 0:H]
                F_J = ff[:, H : 2 * H]
                F_IJ = ff[:, 4 * H : 5 * H]

                sc = ccs.tile([96, 6 * H], BF16, tag="sc", name="sc")
                t1_ = sc[:, 0:H]
                s12 = sc[:, H : 2 * H]
                sg1 = sc[:, 2 * H : 3 * H]
                den = sc[:, 4 * H : 5 * H]
                lnf = sc[:, 5 * H : 6 * H]
                sqs = ccs.tile([96, 2 * H], BF16, tag="sqs", name="sqs")
                scf = ccs.tile([96, 3 * H], F32, tag="scf", name="scf")
                s2f = scf[:, 0:H]
                lnn = scf[:, H : 2 * H]
                lnd = scf[:, 2 * H : 3 * H]
                ccout = ccs.tile([96, H], BF16, tag="ccout", name="ccout")

                nc.vector.tensor_tensor(t1_, F_I, F_J, AOT.mult)
                nc.vector.tensor_tensor(s12, F_IJ, t1_, AOT.subtract)
                nc.scalar.activation(s2f, s12, ACTF.Square)
                nc.scalar.activation(sqs[:], ff[:, 0 : 2 * H], ACTF.Square)
                nc.vector.tensor_tensor(
                    sc[:, 2 * H : 4 * H],
                    ff[:, 2 * H : 4 * H],
                    sqs[:],
                    AOT.subtract,
                )
                nc.vector.tensor_tensor(
                    den, sg1, sc[:, 3 * H : 4 * H], AOT.mult
                )
                nc.scalar.activation(lnn, s2f, ACTF.Ln, bias=bias_tiny[0:96, :])
                nc.scalar.activation(lnd, den, ACTF.Ln, bias=bias_eps[0:96, :])
                nc.vector.tensor_tensor(lnf, lnn, lnd, AOT.subtract)
                nc.scalar.activation(
                    ccout[:], lnf, ACTF.Exp,
                    accum_out=acc[:, 2 * oz + fi : 2 * oz + fi + 1],
                )

        for z in range(din):
            h_pass(z)
            oz = z - 8
            if 0 <= oz < dout:
                w_pass(oz)

        accv = accp.tile([96, 1], F32, tag="accv")
        nc.vector.tensor_reduce(accv[:], acc[:], AXL.X, AOT.add)
        nc.sync.dma_start(out_d.ap(), accv[:])

    nc.compile()
    return nc


_PROGRAM_CACHE = {}


def _get_program(din, dout):
    key = (din, dout)
    if key not in _PROGRAM_CACHE:
        _PROGRAM_CACHE[key] = build_program(din, dout)
    return _PROGRAM_CACHE[key]


def kernel(pred, target):
    pred = np.asarray(pred).reshape(D_TOT, H, W).astype(np.float32)
    targ = np.asarray(target).reshape(D_TOT, H, W).astype(np.float32)

    dout = D_TOT // N_CORES
    din = dout + 2 * PAD

    padded_p = np.zeros((D_TOT + 2 * PAD, HE, WE), np.float32)
    padded_t = np.zeros_like(padded_p)
    padded_p[PAD:-PAD, PAD : PAD + H, PAD : PAD + W] = pred
    padded_t[PAD:-PAD, PAD : PAD + H, PAD : PAD + W] = targ

    band = make_consts()
    nc = _get_program(din, dout)

    in_maps = []
    for c in range(N_CORES):
        s = c * dout
        in_maps.append(
            {
                "pred": np.ascontiguousarray(padded_p[s : s + din]),
                "targ": np.ascontiguousarray(padded_t[s : s + din]),
                "band": band,
            }
        )

    res = run_bass_kernel_spmd(nc, in_maps, core_ids=list(range(N_CORES)))
    total = sum(float(r["out"].astype(np.float64).sum()) for r in res.results)
    return np.float32(1.0 - total / float(D_TOT * H * W))



# revision 9
# speedup vs baseline: 3.0044x; 3.0044x over previous
"""NCC loss (local normalized cross-correlation, window 9^3) on 8 Trainium2
NeuronCores.

Reference: 5 channels [I, J, I^2, J^2, IJ] box-filtered (separable 9-tap mean,
SAME zero-pad) over a 192^3 volume; cc = sigma12^2/(sigma1^2*sigma2^2+eps);
output = 1 - mean(cc).

Sharding: depth axis. Core c computes output slices [24c, 24c+24), reading
mean-shifted bf16 inputs for padded slices [24c, 24c+32) of the (+4 both
ends) zero-padded volume. Host pre-applies the -0.5 mean shift (pads become
-0.5, the shifted zero sample), casts to bf16, interleaves targ|pred, and
duplicates the w overlap so rows arrive w-blocked: [t(2), wc(2), 128] where
wc0 = ext w 0..127 and wc1 = ext w 96..223 (last 24 are pad).

Per-core pipeline:
  load  : 4 z-slices per DMA into [h-part, 4, 512] bf16 tiles.
  prep  : squares + cross product -> blocked (wc, ch, 128) channel tiles
          (2 DVE ops/slice/h-tile); ch0/ch1 feed the H matmul from raw.
  H pass: banded matmuls (TensorE) accumulated over slices into PSUM
          (cumsum over D, 10 blocks of 128 = 2.5 banks per h-tile);
          bf16 snapshots to SBUF each slice (DVE for A, ACT for B).
  T pass: one batched x-bar DMA transpose per snapshot tile per z-pair
          (out 3D AP [128, 10 blocks, rows]) -> T2 [w-part, (half,blk,h)].
  W pass: out(oz) = bandW+ @ T[oz+8] + bandW- @ T[oz-1] accumulated in
          PSUM: the D window diff is folded into the matmul.
  cc    : elementwise DVE ops + one Ln (ACT; natural_log set stays
          resident; square/copy are in that set too); lnf stored per oz;
          Exp+accumulate deferred to a tail pass (2 ACT table loads total).
Host: 1 - sum(partials)/192^3.
"""

import sys

import numpy as np

sys.path.insert(0, "/opt/trn_rl_repo")

import contextlib

import concourse.bacc as bacc
import concourse.mybir as mybir
from concourse import tile
from concourse.bass_utils import run_bass_kernel_spmd

F32 = mybir.dt.float32
BF16 = mybir.dt.bfloat16
AOT = mybir.AluOpType
ACTF = mybir.ActivationFunctionType
AXL = mybir.AxisListType

H = 192
W = 192
D_TOT = 192
HE = 200
PAD = 4
N_CORES = 8

HA = 112           # h-tile A: out rows 0..111 (ext rows 4..115)
HB = 80            # h-tile B: out rows 112..191 (ext rows 116..195)
KT = 128           # A contraction rows: ext h 0..127 (uses 0..119)
KB = 88            # B contraction rows: ext h 112..199

NBLK = 10          # (wc, ch) blocks of 128 cols
BLKW = 128
VALW = 104         # valid w cols per block
ROWW = 512         # raw row: t(2) x wc(2) x 128
TW = NBLK * H      # 1920: T2 free size per z half
TP = 480           # W matmul piece width (4 pieces)

ZB = 4             # z slices per input DMA batch
NPAIR = 3          # snapshot pair ring
NT2 = 6            # transposed z-pair ring

BAND_C = 1.0 / 27.0
EPS = float(np.finfo(np.float32).eps)
TINY = float(np.finfo(np.float32).tiny)


def _band(rows, cols, lo, hi, val):
    k = np.arange(rows)[:, None]
    m = np.arange(cols)[None, :]
    return np.where((k - m >= lo) & (k - m <= hi), val, 0.0).astype(np.float32)


def make_consts():
    import ml_dtypes

    # [120, 304]: cols 0:112 = H band; 112:208 = +W band; 208:304 = -W band
    b = np.zeros((120, 304), np.float32)
    b[:, 0:112] = _band(120, 112, 0, 8, BAND_C)
    bw = _band(104, 96, 0, 8, BAND_C)
    b[0:104, 112:208] = bw
    b[0:104, 208:304] = -bw
    return b.astype(ml_dtypes.bfloat16)


def build_program(din, dout, dbg=False):
    assert din == dout + 2 * PAD
    nc = bacc.Bacc(
        "TRN2", target_bir_lowering=False, debug=False, num_devices=N_CORES
    )

    raw_d = nc.dram_tensor(
        "raw", [din, HE, 2, 2, BLKW], BF16, kind="ExternalInput"
    )
    band_d = nc.dram_tensor("band", [120, 304], BF16, kind="ExternalInput")
    out_d = nc.dram_tensor("out", [96, 1], F32, kind="ExternalOutput")
    if dbg:
        dbg_sA = nc.dram_tensor("dbg_sA", [HA, 2, NBLK * BLKW], BF16,
                                kind="ExternalOutput")
        dbg_t2 = nc.dram_tensor("dbg_t2", [128, 2, TW], BF16,
                                kind="ExternalOutput")
        dbg_ff = nc.dram_tensor("dbg_ff", [2, 96, 4 * TP], BF16,
                                kind="ExternalOutput")

    raw = raw_d.ap()

    with tile.TileContext(nc) as tc, contextlib.ExitStack() as ctx:
        consts = ctx.enter_context(tc.tile_pool(name="consts", bufs=1))
        raws = ctx.enter_context(tc.tile_pool(name="raws", bufs=2))
        chans = ctx.enter_context(tc.tile_pool(name="chans", bufs=3))
        snaps = ctx.enter_context(tc.tile_pool(name="snaps", bufs=1))
        t2s = ctx.enter_context(tc.tile_pool(name="t2s", bufs=1))
        ffs = ctx.enter_context(tc.tile_pool(name="ffs", bufs=2))
        ccs = ctx.enter_context(tc.tile_pool(name="ccs", bufs=2))
        accp = ctx.enter_context(tc.tile_pool(name="accp", bufs=1))
        ps_h = ctx.enter_context(tc.tile_pool(name="psh", bufs=1, space="PSUM"))
        ps_w = ctx.enter_context(tc.tile_pool(name="psw", bufs=1, space="PSUM"))

        band = consts.tile([120, 304], BF16, tag="band")
        nc.sync.dma_start(band[:], band_d.ap())
        bandH_A = band[0:120, 0:112]
        bandH_B = band[0:88, 0:80]
        bandW_p = band[0:104, 112:208]
        bandW_n = band[0:104, 208:304]

        bias_tiny = consts.tile([128, 1], F32, tag="bias_tiny")
        nc.vector.memset(bias_tiny[:], TINY)

        acc = accp.tile([96, dout], F32, tag="acc")
        nc.vector.memset(acc[:], 0.0)
        lnf_buf = accp.tile([96, dout, 2 * H], BF16, tag="lnf")

        # H cumsum PSUM: 10 blocks of 128 -> 3 banks per h-tile
        psA = ps_h.tile([HA, 1536], F32, tag="psA")
        psB = ps_h.tile([HB, 1536], F32, tag="psB")
        ps3A = psA.rearrange("p (b w) -> p b w", b=3)
        ps3B = psB.rearrange("p (b w) -> p b w", b=3)

        # W PSUM: 2 x 1 bank, each used twice per oz
        pws = [
            ps_w.tile([96, 512], F32, tag=f"pw{i}", name=f"pw{i}")
            for i in range(2)
        ]

        # persistent snapshot pair tiles (memset once: pad cols stay 0)
        sAp, sBp = [], []
        for i in range(NPAIR):
            a = snaps.tile([HA, 2, NBLK * BLKW], BF16, tag=f"sAp{i}",
                           name=f"sAp{i}")
            b = snaps.tile([HB, 2, NBLK * BLKW], BF16, tag=f"sBp{i}",
                           name=f"sBp{i}")
            nc.vector.memset(a[:], 0.0)
            nc.vector.memset(b[:], 0.0)
            sAp.append(a)
            sBp.append(b)

        # persistent transposed tiles [w 128, (half, blk, h)]
        T2 = [
            t2s.tile([128, 2, TW], BF16, tag=f"T2_{i}", name=f"T2_{i}")
            for i in range(NT2)
        ]

        raw_tiles = {}

        def load_batch(zb):
            rT = raws.tile([KT, ZB, ROWW], BF16, tag="rT", name="rT")
            rB = raws.tile([KB, ZB, ROWW], BF16, tag="rB", name="rB")
            z0 = zb * ZB
            nc.sync.dma_start(
                rT[:],
                raw[z0 : z0 + ZB, 0:KT].rearrange(
                    "z h t wc w -> h z t wc w"
                ),
            )
            nc.sync.dma_start(
                rB[:],
                raw[z0 : z0 + ZB, HE - KB : HE].rearrange(
                    "z h t wc w -> h z t wc w"
                ),
            )
            raw_tiles[zb] = (rT, rB)

        def prep(z):
            rT, rB = raw_tiles[z // ZB]
            zs = z % ZB
            cT = chans.tile([KT, 6 * BLKW], BF16, tag="cT", name="cT")
            cB = chans.tile([KB, 6 * BLKW], BF16, tag="cB", name="cB")
            for ch, r in ((cT, rT), (cB, rB)):
                # [p, wc, t, w] view of this z slice
                rv = r[:, zs, :].rearrange(
                    "p (t wc w) -> p wc t w", t=2, wc=2
                )
                c4 = ch.rearrange("p (wc b w) -> p wc b w", wc=2, b=3)
                # ch2 = targ^2, ch3 = pred^2 (both wc at once)
                nc.vector.tensor_tensor(
                    c4[:, :, 0:2, 0:VALW],
                    rv[:, :, :, 0:VALW],
                    rv[:, :, :, 0:VALW],
                    AOT.mult,
                )
                # ch4 = targ * pred
                nc.vector.tensor_tensor(
                    c4[:, :, 2, 0:VALW],
                    rv[:, :, 0, 0:VALW],
                    rv[:, :, 1, 0:VALW],
                    AOT.mult,
                )
            return cT, cB

        def h_pass(z, cT, cB):
            # One matmul per PSUM bank: start=True clears has_written at
            # bank granularity, so each bank must be a single accum group.
            # Resulting block order: 0:(t,wc0) 1:(t,wc1) 2:(p,wc0) 3:(p,wc1)
            # 4:(t2,wc0) 5:(p2,wc0) 6:(tp,wc0) 7:(t2,wc1) 8:(p2,wc1) 9:(tp,wc1)
            rT, rB = raw_tiles[z // ZB]
            zs = z % ZB
            start = z == 0
            kw = dict(start=start, stop=True, skip_group_check=True)
            mm = nc.tensor.matmul
            for (ps3, bH, r, ch, kk) in (
                (ps3A, bandH_A, rT, cT, 120),
                (ps3B, bandH_B, rB, cB, 88),
            ):
                rv = r[:, zs, :].rearrange(
                    "p (t wc w) -> p t wc w", t=2, wc=2
                )
                mm(ps3[:, 0, 0:512], bH, rv[0:kk], **kw)
                mm(ps3[:, 1, 0:512], bH, ch[0:kk, 0:512], **kw)
                mm(ps3[:, 2, 0:256], bH, ch[0:kk, 512:768], **kw)

        def snapshot(z):
            half = z % 2
            pa = sAp[(z // 2) % NPAIR]
            pb = sBp[(z // 2) % NPAIR]
            srcA = psA.rearrange("p (b w) -> p b w", b=12)
            srcB = psB.rearrange("p (b w) -> p b w", b=12)
            dA = pa.rearrange("p h (b w) -> p h b w", b=NBLK)
            dB = pb.rearrange("p h (b w) -> p h b w", b=NBLK)
            nc.vector.tensor_copy(
                dA[:, half, :, 0:VALW], srcA[:, 0:NBLK, 0:VALW]
            )
            nc.scalar.copy(dB[:, half, :, 0:VALW], srcB[:, 0:NBLK, 0:VALW])

        def transpose_pair(z):
            # after snapshot of odd z: pair (z//2) holds z-1, z
            p = z // 2
            t2 = T2[p % NT2]
            pa, pb = sAp[p % NPAIR], sBp[p % NPAIR]
            t4 = t2.rearrange("p h (b q) -> p h b q", b=NBLK)
            for half in range(2):
                nc.sync.dma_start_transpose(
                    t4[:, half, :, 0:HA], pa[:, half, :]
                )
                nc.sync.dma_start_transpose(
                    t4[:, half, :, HA:H], pb[:, half, :]
                )

        def w_pass(oz):
            zh = oz + 8
            hi = T2[(zh // 2) % NT2][0:104, zh % 2, :]
            lo = None
            if oz > 0:
                zl = oz - 1
                lo = T2[(zl // 2) % NT2][0:104, zl % 2, :]

            ff = ffs.tile([96, 4 * TP], BF16, tag="ff", name="ff")
            for pp in range(2):        # piece pairs (0,1) then (2,3)
                for q in range(2):
                    sl = slice((2 * pp + q) * TP, (2 * pp + q + 1) * TP)
                    nc.tensor.matmul(
                        pws[q][:, 0:TP], bandW_p, hi[:, sl],
                        start=True, stop=lo is None,
                    )
                if lo is not None:
                    for q in range(2):
                        sl = slice((2 * pp + q) * TP, (2 * pp + q + 1) * TP)
                        nc.tensor.matmul(
                            pws[q][:, 0:TP], bandW_n, lo[:, sl],
                            start=False, stop=True,
                        )
                sl0 = slice(2 * pp * TP, (2 * pp + 1) * TP)
                sl1 = slice((2 * pp + 1) * TP, (2 * pp + 2) * TP)
                nc.vector.tensor_copy(ff[:, sl0], pws[0][:, 0:TP])
                nc.scalar.copy(ff[:, sl1], pws[1][:, 0:TP])

            if dbg and oz <= 1:
                nc.sync.dma_start(dbg_ff.ap()[oz], ff[:])
            cc(oz, ff)

        def cc(oz, ff):
            # block order (see h_pass): mu pair = blocks 0..3 (ch-major,
            # wc-minor); conv trio = blocks 4..9 (wc-major, ch-minor)
            mus = ff[:, 0 : 4 * H].rearrange("p (c wc h) -> p c wc h", c=2,
                                             wc=2)
            cnv = ff[:, 4 * H :].rearrange("p (wc c h) -> p c wc h", wc=2,
                                           c=3)
            mu1 = mus[:, 0]
            mu2 = mus[:, 1]
            cIJ = cnv[:, 2]
            sc = ccs.tile([96, 2, 2, H], BF16, tag="sc", name="sc")
            var = ccs.tile([96, 2, 2, H], BF16, tag="var", name="var")
            lnp = ccs.tile([96, 2, 2, H], BF16, tag="lnp", name="lnp")
            lno = ccs.tile([96, 2, 2, H], BF16, tag="lno", name="lno")
            sqs = ccs.tile([96, 2, 2, H], BF16, tag="sqs", name="sqs")
            t1 = sc[:, 0]
            s12 = sc[:, 1]

            # t1 = mu1*mu2 ; s12 = F_IJ - t1
            nc.vector.tensor_tensor(t1, mu1, mu2, AOT.mult)
            nc.vector.tensor_tensor(s12, cIJ, t1, AOT.subtract)
            # sqs = [mu1^2, mu2^2] ; var = conv - sqs   (dims (ch, wc))
            nc.vector.tensor_tensor(sqs[:], mus[:], mus[:], AOT.mult)
            nc.vector.tensor_tensor(var[:], cnv[:, 0:2], sqs[:],
                                    AOT.subtract)
            # lnp = [s12^2 | sg1*sg2]
            nc.vector.tensor_tensor(lnp[:, 0], s12, s12, AOT.mult)
            nc.vector.tensor_tensor(lnp[:, 1], var[:, 0], var[:, 1],
                                    AOT.mult)
            nc.scalar.activation(lno[:], lnp[:], ACTF.Ln,
                                 bias=bias_tiny[0:96, :])
            lv = lnf_buf[:, oz, :].rearrange("p (wc h) -> p wc h", wc=2)
            nc.vector.tensor_tensor(lv, lno[:, 0], lno[:, 1], AOT.subtract)

        for z in range(din):
            if z % ZB == 0:
                load_batch(z // ZB)
            cT, cB = prep(z)
            h_pass(z, cT, cB)
            snapshot(z)
            if z % 2 == 1:
                transpose_pair(z)
                if dbg and z == 1:
                    nc.sync.dma_start(dbg_sA.ap(), sAp[0][:])
                    nc.sync.dma_start(dbg_t2.ap(), T2[0][:])
                if z >= 9:
                    w_pass(z - 9)
                    w_pass(z - 8)

        # tail: all Exps (one table switch), accumulate per-oz sums
        ccout = accp.tile([96, 2 * H], BF16, tag="ccout")
        for oz in range(dout):
            nc.scalar.activation(
                ccout[:], lnf_buf[:, oz, :], ACTF.Exp,
                accum_out=acc[:, oz : oz + 1],
            )
        accv = accp.tile([96, 1], F32, tag="accv")
        nc.vector.tensor_reduce(accv[:], acc[:], AXL.X, AOT.add)
        nc.sync.dma_start(out_d.ap(), accv[:])

    nc.compile()
    return nc


_PROGRAM_CACHE = {}


def _get_program(din, dout):
    key = (din, dout)
    if key not in _PROGRAM_CACHE:
        _PROGRAM_CACHE[key] = build_program(din, dout)
    return _PROGRAM_CACHE[key]


def make_in_maps(pred, targ):
    """Build per-core input maps from full 192^3 f32 volumes."""
    import ml_dtypes

    dout = D_TOT // N_CORES
    din = dout + 2 * PAD

    # shifted, padded volume rows: [dpad, 200, 2, 224] bf16, pads = -0.5
    dpad = D_TOT + 2 * PAD
    we = np.full((dpad, HE, 2, 224), -0.5, np.float32)
    we[PAD:-PAD, PAD : PAD + H, 0, PAD : PAD + W] = targ - 0.5
    we[PAD:-PAD, PAD : PAD + H, 1, PAD : PAD + W] = pred - 0.5
    # w-blocked rows: [dpad, 200, 2, 2, 128]
    arr = np.empty((dpad, HE, 2, 2, BLKW), np.float32)
    arr[:, :, :, 0, :] = we[:, :, :, 0:128]
    arr[:, :, :, 1, :] = we[:, :, :, 96:224]
    arr = arr.astype(ml_dtypes.bfloat16)

    band = make_consts()
    in_maps = []
    for c in range(N_CORES):
        s = c * dout
        in_maps.append(
            {
                "raw": np.ascontiguousarray(arr[s : s + din]),
                "band": band,
            }
        )
    return in_maps


def kernel(pred, target):
    pred = np.asarray(pred).reshape(D_TOT, H, W).astype(np.float32)
    targ = np.asarray(target).reshape(D_TOT, H, W).astype(np.float32)

    dout = D_TOT // N_CORES
    din = dout + 2 * PAD

    nc = _get_program(din, dout)
    in_maps = make_in_maps(pred, targ)

    res = run_bass_kernel_spmd(nc, in_maps, core_ids=list(range(N_CORES)))
    total = sum(float(r["out"].astype(np.float64).sum()) for r in res.results)
    return np.float32(1.0 - total / float(D_TOT * H * W))
